# revision 15
# baseline (speedup 1.0000x reference)
"""Trainium2 Bass kernel for nn_EnhancedSpatialAttention.

Row-sharded across 8 NeuronCores: core c owns query rows [c*512, (c+1)*512).
Each core computes its rows of phi [512,4096,4] and attn [512,4096] on
device; attended = attn @ h (tiny [4096,16]) is a host epilogue on the
gathered attn. Total device I/O is ~52.4 MB/core, so the kernel targets the
~358 GB/s-per-core HBM roofline (~147 us).

All pairwise quantities are bilinear forms computed on the PE in float32r
(1 cycle/row). float32r rounds operands to 13 mantissa bits, so the
position/velocity operands are split hi/lo (hi keeps 11 bits => exactly
representable; lo rounds with ~2^-24 relative error) and the three O(x*y)
cross terms are packed into a single K=10 contraction:
  dist^2 = 1*|pj|^2(hi+lo) - 2x_i*x_j - 2y_i*y_j + |pi|^2(hi+lo)*1
Measured accuracy: max_abs ~5e-4 over values up to ~5000 (q99 rel 2e-6),
same as a full fp32 matmul, 4x faster.

Phase A per row-tile rt (128 rows) x 1024-column chunk:
  PE (float32r): psum_d (K=10), psum_v (K=10), psum_a = dir_i.dir_j (K=2),
      psum_b = li + lj + ws2*(dir_i.dir_j) (K=4, ws2 folded into lhsT)
  ACT: phi plane 0 = sqrt(psum_d), plane 1 = sqrt(psum_v). A tiny +delta
       K-row (2e-3 / 1e-4) keeps the psums strictly positive against the
       ~1e-3 fp32 cancellation residue near zero distances - ACT sqrt of
       any negative is NaN (probed). The diagonal is re-zeroed on the host.
  DVE: phi plane 2 = clip(psum_a, -1, 1)
       logits = (phi0*ws0) + psum_b ; logits += phi1*ws1
       conflict cols [3072:4096]: logits += ws3*conf_rows
  ACT: conf_rows -> phi plane 3 (cols 3072:4096; rest stays memset 0)
       logits = prelu(logits, 0.2)  (probed: Prelu honors alpha, Lrelu not)
  DMA: phi out planar [128, 4, 1024] chunks; host transposes [V,4,V] ->
       [V,V,4] (0.2 s) so every engine op reads/writes dense unit-stride

Phase B per row-tile:
  ACT: e = exp(logits + (-M)); M = host per-row upper bound (exact softmax:
       the exp scale cancels in w/sum(w))
  DVE: w = e*adj with fused row-sum (scalar_tensor_tensor accum_out)
       attn = w * reciprocal(wsum) -> DMA out

Softmax equivalence with the reference (softmax(logits+mask)*adj then
renormalized with +1e-8): masked entries are exactly 0 both ways, and the
+1e-8 terms are below fp32 resolution of the O(1) row sums, so both reduce
to e*adj / sum(e*adj).
"""
import sys, os

if "/opt/trn_rl_repo" not in sys.path:
    sys.path.insert(0, "/opt/trn_rl_repo")

import numpy as np

V = 4096
H = 16
G = 1024
N_CORES = 8
ROWS = V // N_CORES          # 512 rows per core
RT = ROWS // 128             # 4 row-tiles per core
EPS = 1e-8
SLOPE = 0.2

_cache = {}


def _split_hi_lo(x):
    """hi keeps 11 explicit mantissa bits (exact in 13-bit float32r)."""
    x = np.asarray(x, np.float32)
    hi = (x.view(np.uint32) & np.uint32(0xFFFFF000)).view(np.float32)
    return hi, (x - hi).astype(np.float32)


def _build_program():
    import concourse.bass as bass
    import concourse.bacc as bacc
    import concourse.tile as tile
    from concourse import mybir
    from contextlib import ExitStack

    f32 = mybir.dt.float32
    f32r = mybir.dt.float32r
    AF = mybir.ActivationFunctionType
    OP = mybir.AluOpType

    nc = bacc.Bacc("TRN2", target_bir_lowering=False, debug=False)

    # d-quantity rows 0:10 | v rows 32:42 | a rows 64:66 (matmul operands
    # must start at base partition 0/32/64)
    fdva_dram = nc.dram_tensor("feats_dva", [96, V], f32, kind="ExternalInput")
    fb_dram = nc.dram_tensor("featsb", [4, V], f32, kind="ExternalInput")
    ldva_dram = nc.dram_tensor("lhsT_dva", [96, ROWS], f32, kind="ExternalInput")
    lb_dram = nc.dram_tensor("lhsTb", [4, ROWS], f32, kind="ExternalInput")
    conf_dram = nc.dram_tensor("conf_rows", [ROWS, G], f32, kind="ExternalInput")
    adj_dram = nc.dram_tensor("adj_rows", [ROWS, V], f32, kind="ExternalInput")
    negM_dram = nc.dram_tensor("negM", [128, RT], f32, kind="ExternalInput")
    ws_dram = nc.dram_tensor("ws4", [128, 4], f32, kind="ExternalInput")

    phi_dram = nc.dram_tensor("phi_out", [ROWS, 4, V], f32, kind="ExternalOutput")
    attn_dram = nc.dram_tensor("attn_out", [ROWS, V], f32, kind="ExternalOutput")

    CH = 1024                    # column chunk == phi quarter width
    NCH = V // CH

    with tile.TileContext(nc) as tc:
        with ExitStack() as ctx:
            persist = ctx.enter_context(tc.tile_pool(name="persist", bufs=1))
            psum_pool = ctx.enter_context(
                tc.tile_pool(name="psum", bufs=1, space="PSUM"))

            logits = [persist.tile([128, V], f32, tag=f"logits{rt}",
                                   name=f"logits{rt}") for rt in range(RT)]
            negM = persist.tile([128, RT], f32, tag="negM", name="negM")
            ws4 = persist.tile([128, 4], f32, tag="ws4", name="ws4")
            nc.sync.dma_start(negM[:], negM_dram[:])
            nc.sync.dma_start(ws4[:], ws_dram[:])

            # ---------------- Phase A ----------------
            with ExitStack() as ctxA:
                pa = ctxA.enter_context(tc.tile_pool(name="pa", bufs=1))
                confp = ctxA.enter_context(tc.tile_pool(name="confp", bufs=2))

                fdva = pa.tile([96, V], f32r, tag="fdva", name="fdva")
                fb = pa.tile([4, V], f32r, tag="fb", name="fb")
                ldva = pa.tile([96, ROWS], f32r, tag="ldva", name="ldva")
                lb = pa.tile([4, ROWS], f32r, tag="lb", name="lb")
                nc.gpsimd.dma_start(fdva[:], fdva_dram[:])
                nc.gpsimd.dma_start(ldva[:], ldva_dram[:])
                nc.gpsimd.dma_start(fb[:], fb_dram[:])
                nc.gpsimd.dma_start(lb[:], lb_dram[:])

                # phi quarter buffers: even chunks -> phiA, odd -> phiB.
                # lane 3 of phiA stays all-zero; phiB's lane 3 is rewritten
                # with conf on chunk 3 and re-zeroed before chunk 1 reuses it.
                phiA = pa.tile([128, 4, CH], f32, tag="phiA", name="phiA")
                phiB = pa.tile([128, 4, CH], f32, tag="phiB", name="phiB")
                nc.vector.memset(phiA[:, 3, :], 0.0)

                for rt in range(RT):
                    sA = logits[rt]
                    rsl = slice(rt * 128, (rt + 1) * 128)
                    conf_t = confp.tile([128, G], f32, tag="conf", name="conf")
                    nc.scalar.dma_start(conf_t[:], conf_dram[rsl, :])
                    for c in range(NCH):
                        quarter = phiA if c % 2 == 0 else phiB
                        if c == 1:
                            # clear stale conflict values from last rt's c==3
                            nc.gpsimd.memset(phiB[:, 3, :], 0.0)
                        ps_d = psum_pool.tile([128, CH], f32, tag="psd", name="psd")
                        ps_v = psum_pool.tile([128, CH], f32, tag="psv", name="psv")
                        ps_a = psum_pool.tile([128, CH], f32, tag="psa", name="psa")
                        ps_b = psum_pool.tile([128, CH], f32, tag="psb", name="psb")
                        for s in range(2):
                            cs = slice(c * CH + s * 512, c * CH + s * 512 + 512)
                            half = slice(s * 512, s * 512 + 512)
                            nc.tensor.matmul(ps_d[:, half], ldva[0:11, rsl],
                                             fdva[0:11, cs], start=True, stop=True)
                            nc.tensor.matmul(ps_v[:, half], ldva[32:43, rsl],
                                             fdva[32:43, cs], start=True, stop=True)
                            nc.tensor.matmul(ps_a[:, half], ldva[64:66, rsl],
                                             fdva[64:66, cs], start=True, stop=True)
                            nc.tensor.matmul(ps_b[:, half], lb[:, rsl],
                                             fb[:, cs], start=True, stop=True)
                        lane = lambda k: quarter[:, k, :]
                        nc.scalar.activation(lane(0), ps_d[:], AF.Sqrt)
                        nc.scalar.activation(lane(1), ps_v[:], AF.Sqrt)
                        nc.vector.tensor_scalar(
                            out=lane(2), in0=ps_a[:], scalar1=1.0, scalar2=-1.0,
                            op0=OP.min, op1=OP.max)
                        cols = slice(c * CH, (c + 1) * CH)
                        nc.vector.scalar_tensor_tensor(
                            out=sA[:, cols], in0=lane(0),
                            scalar=ws4[:, 0:1], in1=ps_b[:],
                            op0=OP.mult, op1=OP.add)
                        nc.vector.scalar_tensor_tensor(
                            out=sA[:, cols], in0=lane(1),
                            scalar=ws4[:, 1:2], in1=sA[:, cols],
                            op0=OP.mult, op1=OP.add)
                        if c == 3:
                            # conflict block lives in cols [V-G, V) == chunk 3
                            nc.scalar.activation(lane(3), conf_t[:], AF.Copy)
                            nc.vector.scalar_tensor_tensor(
                                out=sA[:, cols], in0=conf_t[:],
                                scalar=ws4[:, 3:4], in1=sA[:, cols],
                                op0=OP.mult, op1=OP.add)
                        r0 = rt * 128
                        nc.sync.dma_start(
                            phi_dram[r0:r0 + 128, :, c * CH:(c + 1) * CH],
                            quarter[:])
                    # leaky relu in place (Prelu honors alpha; Lrelu's is baked)
                    nc.scalar.activation(
                        logits[rt][:], logits[rt][:], AF.Prelu, alpha=SLOPE)

            # ---------------- Phase B ----------------
            with ExitStack() as ctxB:
                pb = ctxB.enter_context(tc.tile_pool(name="pb", bufs=2))
                for rt in range(RT):
                    r0 = rt * 128
                    adj_t = pb.tile([128, V], f32, tag="adj", name="adj_t")
                    nc.scalar.dma_start(adj_t[:], adj_dram[r0:r0 + 128, :])
                    e_t = pb.tile([128, V], f32, tag="e", name="e_t", bufs=3)
                    nc.scalar.activation(
                        e_t[:], logits[rt][:], AF.Exp,
                        bias=negM[:, rt:rt + 1], scale=1.0)
                    w_t = pb.tile([128, V], f32, tag="w", name="w_t")
                    wsum = pb.tile([128, 1], f32, tag="wsum", name="wsum")
                    nc.vector.scalar_tensor_tensor(
                        out=w_t[:], in0=e_t[:], scalar=1.0, in1=adj_t[:],
                        op0=OP.mult, op1=OP.mult, accum_out=wsum[:])
                    rcp = pb.tile([128, 1], f32, tag="rcp", name="rcp")
                    nc.vector.reciprocal(rcp[:], wsum[:])
                    nc.vector.tensor_scalar_mul(
                        out=w_t[:], in0=w_t[:], scalar1=rcp[:])
                    nc.sync.dma_start(attn_dram[r0:r0 + 128, :], w_t[:])

    nc.compile()
    return nc


def _host_prep(node_features, adjacency, node_positions, node_velocities,
               node_types, num_ped_nodes, conflict_softmax_group,
               W_proj, w_score):
    nf = np.asarray(node_features, np.float32)
    adj = np.ascontiguousarray(np.asarray(adjacency, np.float32))
    pos = np.asarray(node_positions, np.float32)
    vel = np.asarray(node_velocities, np.float32)
    types = np.asarray(node_types)
    P = int(num_ped_nodes)
    conf = np.asarray(conflict_softmax_group, np.float32)
    Wp = np.asarray(W_proj, np.float32)
    ws = np.asarray(w_score, np.float32)

    h = nf @ Wp                                     # [V, H] fp32
    li = h @ ws[:H]
    lj = h @ ws[H:2 * H]
    speed = np.sqrt((vel ** 2).sum(-1, keepdims=True))
    dirv = vel / (speed + EPS)
    ws4 = ws[2 * H:2 * H + 4]

    p2 = (pos.astype(np.float64) ** 2).sum(-1).astype(np.float32)
    v2 = (vel.astype(np.float64) ** 2).sum(-1).astype(np.float32)
    p2h, p2l = _split_hi_lo(p2)
    v2h, v2l = _split_hi_lo(v2)
    xh, xl = _split_hi_lo(pos[:, 0])
    yh, yl = _split_hi_lo(pos[:, 1])
    vxh, vxl = _split_hi_lo(vel[:, 0])
    vyh, vyl = _split_hi_lo(vel[:, 1])

    # K=10 hi/lo layout (feats side / lhsT side):
    #  [q2_hi, q2_lo, xh, xl, xh, yh, yl, yh, 1, 1]
    #  [1, 1, -2xh_i, -2xh_i, -2xl_i, -2yh_i, -2yh_i, -2yl_i, q2_hi, q2_lo]
    fdva = np.zeros((96, V), np.float32)
    fdva[0] = p2h; fdva[1] = p2l
    fdva[2] = xh;  fdva[3] = xl;  fdva[4] = xh
    fdva[5] = yh;  fdva[6] = yl;  fdva[7] = yh
    fdva[8] = 1.0; fdva[9] = 1.0
    fdva[10] = 1.0
    fdva[32] = v2h; fdva[33] = v2l
    fdva[34] = vxh; fdva[35] = vxl; fdva[36] = vxh
    fdva[37] = vyh; fdva[38] = vyl; fdva[39] = vyh
    fdva[40] = 1.0; fdva[41] = 1.0
    fdva[42] = 1.0
    fdva[64] = dirv[:, 0]; fdva[65] = dirv[:, 1]
    fb = np.zeros((4, V), np.float32)
    fb[0] = 1.0
    fb[1] = lj
    fb[2] = dirv[:, 0]
    fb[3] = dirv[:, 1]

    g = np.arange(V) - P
    valid = (types == 1) & (g >= 0) & (g < G)
    gc = np.clip(g, 0, G - 1)
    jcols = np.arange(V - G, V)
    gc_j = np.clip(jcols - P, 0, G - 1)
    valid_j = valid[jcols]

    # per-row upper bound on post-leaky logits (fp64, provably >= rowmax)
    p64 = pos.astype(np.float64)
    v64 = vel.astype(np.float64)
    pn = np.sqrt((p64 ** 2).sum(-1))
    vn = np.sqrt((v64 ** 2).sum(-1))
    t_dist = np.maximum(0.0, float(ws4[0]) * (pn + pn.max()))
    t_vel = np.maximum(0.0, float(ws4[1]) * (vn + vn.max()))
    m_pre = (li.astype(np.float64) + lj.astype(np.float64).max()
             + t_dist + t_vel + abs(float(ws4[2]))
             + max(float(ws4[3]), 0.0))
    m_bound = np.where(m_pre >= 0, m_pre, SLOPE * m_pre) + 0.5

    ws4_tile = np.ascontiguousarray(
        np.broadcast_to(ws4[None, :], (128, 4)).astype(np.float32))

    in_maps = []
    for c in range(N_CORES):
        rows = np.arange(c * ROWS, (c + 1) * ROWS)
        ldva = np.zeros((96, ROWS), np.float32)
        ldva[0] = 1.0; ldva[1] = 1.0
        ldva[2] = -2.0 * xh[rows]; ldva[3] = -2.0 * xh[rows]
        ldva[4] = -2.0 * xl[rows]
        ldva[5] = -2.0 * yh[rows]; ldva[6] = -2.0 * yh[rows]
        ldva[7] = -2.0 * yl[rows]
        ldva[8] = p2h[rows]; ldva[9] = p2l[rows]
        ldva[10] = 2e-3
        ldva[32] = 1.0; ldva[33] = 1.0
        ldva[34] = -2.0 * vxh[rows]; ldva[35] = -2.0 * vxh[rows]
        ldva[36] = -2.0 * vxl[rows]
        ldva[37] = -2.0 * vyh[rows]; ldva[38] = -2.0 * vyh[rows]
        ldva[39] = -2.0 * vyl[rows]
        ldva[40] = v2h[rows]; ldva[41] = v2l[rows]
        ldva[42] = 1e-4
        ldva[64] = dirv[rows, 0]; ldva[65] = dirv[rows, 1]
        lb = np.zeros((4, ROWS), np.float32)
        lb[0] = li[rows]
        lb[1] = 1.0
        lb[2] = float(ws4[2]) * dirv[rows, 0]
        lb[3] = float(ws4[2]) * dirv[rows, 1]

        conf_rows = (conf[gc[rows]][:, gc_j]
                     * valid[rows][:, None] * valid_j[None, :]).astype(np.float32)
        negM = np.ascontiguousarray(
            (-m_bound[rows]).astype(np.float32).reshape(RT, 128).T)
        in_maps.append({
            "feats_dva": fdva,
            "featsb": fb,
            "lhsT_dva": ldva,
            "lhsTb": lb,
            "conf_rows": np.ascontiguousarray(conf_rows),
            "adj_rows": np.ascontiguousarray(adj[rows]),
            "negM": negM,
            "ws4": ws4_tile,
        })
    return in_maps, h, valid, gc, P


def kernel(**inputs):
    from concourse.bass_utils import run_bass_kernel_spmd

    if "nc" not in _cache:
        _cache["nc"] = _build_program()
    nc = _cache["nc"]

    in_maps, h, valid, gc, P = _host_prep(**inputs)

    trace = os.environ.get("KERNEL_TRACE", "0") == "1"
    res = run_bass_kernel_spmd(
        nc, in_maps, core_ids=list(range(N_CORES)), trace=trace)
    _cache["last_results"] = res

    attn = np.concatenate([r["attn_out"] for r in res.results], axis=0)
    phi_pl = np.concatenate([r["phi_out"] for r in res.results], axis=0)
    phi = np.ascontiguousarray(phi_pl.transpose(0, 2, 1))

    # exact-zero diagonal (reference computes p_i - p_i = 0 exactly; the
    # bilinear-form route leaves ~sqrt(cancellation) residue there)
    idx = np.arange(V)
    phi[idx, idx, 0] = 0.0
    phi[idx, idx, 1] = 0.0

    if P != V - G:  # defensive: conflict block position hardcoded for P=3072
        conf = np.asarray(inputs["conflict_softmax_group"], np.float32)
        nconf = np.where(valid[:, None] & valid[None, :],
                         conf[gc[:, None], gc[None, :]], 0.0)
        phi[:, :, 3] = nconf

    attended = (attn @ h).astype(np.float32)
    return attended, attn, phi


# revision 16
# speedup vs baseline: 1.0333x; 1.0333x over previous
"""Trainium2 Bass kernel for nn_EnhancedSpatialAttention.

Row-sharded across 8 NeuronCores: core c owns query rows [c*512, (c+1)*512).
Each core computes its rows of phi [512,4096,4] and attn [512,4096] on
device; attended = attn @ h (tiny [4096,16]) is a host epilogue on the
gathered attn. Total device I/O is ~52.4 MB/core, so the kernel targets the
~358 GB/s-per-core HBM roofline (~147 us).

All pairwise quantities are bilinear forms computed on the PE in float32r
(1 cycle/row). float32r rounds operands to 13 mantissa bits, so the
position/velocity operands are split hi/lo (hi keeps 11 bits => exactly
representable; lo rounds with ~2^-24 relative error) and the three O(x*y)
cross terms are packed into a single K=10 contraction:
  dist^2 = 1*|pj|^2(hi+lo) - 2x_i*x_j - 2y_i*y_j + |pi|^2(hi+lo)*1
Measured accuracy: max_abs ~5e-4 over values up to ~5000 (q99 rel 2e-6),
same as a full fp32 matmul, 4x faster.

Phase A per row-tile rt (128 rows) x 1024-column chunk:
  PE (float32r): psum_d (K=10), psum_v (K=10), psum_a = dir_i.dir_j (K=2),
      psum_b = li + lj + ws2*(dir_i.dir_j) (K=4, ws2 folded into lhsT)
  ACT: phi plane 0 = sqrt(psum_d), plane 1 = sqrt(psum_v). A tiny +delta
       K-row (2e-3 / 1e-4) keeps the psums strictly positive against the
       ~1e-3 fp32 cancellation residue near zero distances - ACT sqrt of
       any negative is NaN (probed). The diagonal is re-zeroed on the host.
  DVE: phi plane 2 = clip(psum_a, -1, 1)
       logits = (phi0*ws0) + psum_b ; logits += phi1*ws1
       conflict cols [3072:4096]: logits += ws3*conf_rows
  ACT: conf_rows -> phi plane 3 (cols 3072:4096; rest stays memset 0)
       logits = prelu(logits, 0.2)  (probed: Prelu honors alpha, Lrelu not)
  DMA: phi out planar [128, 4, 1024] chunks; host transposes [V,4,V] ->
       [V,V,4] (0.2 s) so every engine op reads/writes dense unit-stride

Phase B per row-tile:
  ACT: e = exp(logits + (-M)); M = host per-row upper bound (exact softmax:
       the exp scale cancels in w/sum(w))
  DVE: w = e*adj with fused row-sum (scalar_tensor_tensor accum_out)
       attn = w * reciprocal(wsum) -> DMA out

Softmax equivalence with the reference (softmax(logits+mask)*adj then
renormalized with +1e-8): masked entries are exactly 0 both ways, and the
+1e-8 terms are below fp32 resolution of the O(1) row sums, so both reduce
to e*adj / sum(e*adj).
"""
import sys, os

if "/opt/trn_rl_repo" not in sys.path:
    sys.path.insert(0, "/opt/trn_rl_repo")

import numpy as np

V = 4096
H = 16
G = 1024
N_CORES = 8
ROWS = V // N_CORES          # 512 rows per core
RT = ROWS // 128             # 4 row-tiles per core
EPS = 1e-8
SLOPE = 0.2

_cache = {}


def _split_hi_lo(x):
    """hi keeps 11 explicit mantissa bits (exact in 13-bit float32r)."""
    x = np.asarray(x, np.float32)
    hi = (x.view(np.uint32) & np.uint32(0xFFFFF000)).view(np.float32)
    return hi, (x - hi).astype(np.float32)


def _build_program():
    import concourse.bass as bass
    import concourse.bacc as bacc
    import concourse.tile as tile
    from concourse import mybir
    from contextlib import ExitStack

    f32 = mybir.dt.float32
    f32r = mybir.dt.float32r
    AF = mybir.ActivationFunctionType
    OP = mybir.AluOpType

    nc = bacc.Bacc("TRN2", target_bir_lowering=False, debug=False)

    # d-quantity rows 0:10 | v rows 32:42 | a rows 64:66 (matmul operands
    # must start at base partition 0/32/64)
    fdva_dram = nc.dram_tensor("feats_dva", [96, V], f32, kind="ExternalInput")
    fb_dram = nc.dram_tensor("featsb", [4, V], f32, kind="ExternalInput")
    ldva_dram = nc.dram_tensor("lhsT_dva", [96, ROWS], f32, kind="ExternalInput")
    lb_dram = nc.dram_tensor("lhsTb", [4, ROWS], f32, kind="ExternalInput")
    conf_dram = nc.dram_tensor("conf_rows", [ROWS, G], f32, kind="ExternalInput")
    adj_dram = nc.dram_tensor("adj_rows", [ROWS, V], f32, kind="ExternalInput")
    negM_dram = nc.dram_tensor("negM", [128, RT], f32, kind="ExternalInput")
    ws_dram = nc.dram_tensor("ws4", [128, 4], f32, kind="ExternalInput")

    phi_dram = nc.dram_tensor("phi_out", [ROWS, 4, V], f32, kind="ExternalOutput")
    attn_dram = nc.dram_tensor("attn_out", [ROWS, V], f32, kind="ExternalOutput")

    CH = 1024                    # column chunk == phi quarter width
    NCH = V // CH

    with tile.TileContext(nc) as tc:
        with ExitStack() as ctx:
            persist = ctx.enter_context(tc.tile_pool(name="persist", bufs=1))
            psum_pool = ctx.enter_context(
                tc.tile_pool(name="psum", bufs=1, space="PSUM"))

            logits = [persist.tile([128, V], f32, tag=f"logits{rt}",
                                   name=f"logits{rt}") for rt in range(RT)]
            negM = persist.tile([128, RT], f32, tag="negM", name="negM")
            ws4 = persist.tile([128, 4], f32, tag="ws4", name="ws4")
            nc.sync.dma_start(negM[:], negM_dram[:])
            nc.sync.dma_start(ws4[:], ws_dram[:])

            # ---------------- Phase A ----------------
            with ExitStack() as ctxA:
                pa = ctxA.enter_context(tc.tile_pool(name="pa", bufs=1))
                confp = ctxA.enter_context(tc.tile_pool(name="confp", bufs=2))

                fdva = pa.tile([96, V], f32r, tag="fdva", name="fdva")
                fb = pa.tile([4, V], f32r, tag="fb", name="fb")
                ldva = pa.tile([96, ROWS], f32r, tag="ldva", name="ldva")
                lb = pa.tile([4, ROWS], f32r, tag="lb", name="lb")
                nc.gpsimd.dma_start(fdva[:], fdva_dram[:])
                nc.gpsimd.dma_start(ldva[:], ldva_dram[:])
                nc.gpsimd.dma_start(fb[:], fb_dram[:])
                nc.gpsimd.dma_start(lb[:], lb_dram[:])

                # phi quarter buffers: even chunks -> phiA, odd -> phiB.
                # lane 3 of phiA stays all-zero; phiB's lane 3 is rewritten
                # with conf on chunk 3 and re-zeroed before chunk 1 reuses it.
                phiA = pa.tile([128, 4, CH], f32, tag="phiA", name="phiA")
                phiB = pa.tile([128, 4, CH], f32, tag="phiB", name="phiB")
                nc.vector.memset(phiA[:, 3, :], 0.0)

                for rt in range(RT):
                    sA = logits[rt]
                    rsl = slice(rt * 128, (rt + 1) * 128)
                    conf_t = confp.tile([128, G], f32, tag="conf", name="conf")
                    nc.scalar.dma_start(conf_t[:], conf_dram[rsl, :])
                    for c in range(NCH):
                        quarter = phiA if c % 2 == 0 else phiB
                        if c == 1:
                            # clear stale conflict values from last rt's c==3
                            nc.gpsimd.memset(phiB[:, 3, :], 0.0)
                        ps_d = psum_pool.tile([128, CH], f32, tag="psd", name="psd")
                        ps_v = psum_pool.tile([128, CH], f32, tag="psv", name="psv")
                        ps_a = psum_pool.tile([128, CH], f32, tag="psa", name="psa")
                        ps_b = psum_pool.tile([128, CH], f32, tag="psb", name="psb")
                        for s in range(2):
                            cs = slice(c * CH + s * 512, c * CH + s * 512 + 512)
                            half = slice(s * 512, s * 512 + 512)
                            nc.tensor.matmul(ps_d[:, half], ldva[0:11, rsl],
                                             fdva[0:11, cs], start=True, stop=True)
                            nc.tensor.matmul(ps_v[:, half], ldva[32:43, rsl],
                                             fdva[32:43, cs], start=True, stop=True)
                            nc.tensor.matmul(ps_a[:, half], ldva[64:66, rsl],
                                             fdva[64:66, cs], start=True, stop=True)
                            nc.tensor.matmul(ps_b[:, half], lb[:, rsl],
                                             fb[:, cs], start=True, stop=True)
                        lane = lambda k: quarter[:, k, :]
                        nc.scalar.activation(lane(0), ps_d[:], AF.Sqrt)
                        nc.scalar.activation(lane(1), ps_v[:], AF.Sqrt)
                        nc.vector.tensor_scalar(
                            out=lane(2), in0=ps_a[:], scalar1=1.0, scalar2=-1.0,
                            op0=OP.min, op1=OP.max)
                        cols = slice(c * CH, (c + 1) * CH)
                        nc.vector.scalar_tensor_tensor(
                            out=sA[:, cols], in0=lane(0),
                            scalar=ws4[:, 0:1], in1=ps_b[:],
                            op0=OP.mult, op1=OP.add)
                        nc.vector.scalar_tensor_tensor(
                            out=sA[:, cols], in0=lane(1),
                            scalar=ws4[:, 1:2], in1=sA[:, cols],
                            op0=OP.mult, op1=OP.add)
                        if c == 3:
                            # conflict block lives in cols [V-G, V) == chunk 3
                            nc.scalar.activation(lane(3), conf_t[:], AF.Copy)
                            nc.vector.scalar_tensor_tensor(
                                out=sA[:, cols], in0=conf_t[:],
                                scalar=ws4[:, 3:4], in1=sA[:, cols],
                                op0=OP.mult, op1=OP.add)
                        r0 = rt * 128
                        nc.sync.dma_start(
                            phi_dram[r0:r0 + 128, :, c * CH:(c + 1) * CH],
                            quarter[:])
                    # leaky relu in place (Prelu honors alpha; Lrelu's is baked)
                    nc.scalar.activation(
                        logits[rt][:], logits[rt][:], AF.Prelu, alpha=SLOPE)

            # ---------------- Phase B ----------------
            with ExitStack() as ctxB:
                pb = ctxB.enter_context(tc.tile_pool(name="pb", bufs=2))
                HV = V // 2
                for rt in range(RT):
                    r0 = rt * 128
                    adj_t = pb.tile([128, V], f32, tag="adj", name="adj_t")
                    e_t = pb.tile([128, V], f32, tag="e", name="e_t")
                    w_t = pb.tile([128, V], f32, tag="w", name="w_t")
                    wsum_h = pb.tile([128, 2], f32, tag="wsumh", name="wsum_h")
                    for hh in range(2):
                        cl = slice(hh * HV, (hh + 1) * HV)
                        nc.scalar.dma_start(
                            adj_t[:, cl], adj_dram[r0:r0 + 128, cl])
                        nc.scalar.activation(
                            e_t[:, cl], logits[rt][:, cl], AF.Exp,
                            bias=negM[:, rt:rt + 1], scale=1.0)
                        nc.vector.scalar_tensor_tensor(
                            out=w_t[:, cl], in0=e_t[:, cl], scalar=1.0,
                            in1=adj_t[:, cl], op0=OP.mult, op1=OP.mult,
                            accum_out=wsum_h[:, hh:hh + 1])
                    wsum = pb.tile([128, 1], f32, tag="wsum", name="wsum")
                    nc.vector.tensor_tensor(
                        out=wsum[:], in0=wsum_h[:, 0:1], in1=wsum_h[:, 1:2],
                        op=OP.add)
                    rcp = pb.tile([128, 1], f32, tag="rcp", name="rcp")
                    nc.vector.reciprocal(rcp[:], wsum[:])
                    for hh in range(2):
                        cl = slice(hh * HV, (hh + 1) * HV)
                        nc.vector.tensor_scalar_mul(
                            out=w_t[:, cl], in0=w_t[:, cl], scalar1=rcp[:])
                        nc.sync.dma_start(
                            attn_dram[r0:r0 + 128, cl], w_t[:, cl])

    nc.compile()
    return nc


def _host_prep(node_features, adjacency, node_positions, node_velocities,
               node_types, num_ped_nodes, conflict_softmax_group,
               W_proj, w_score):
    nf = np.asarray(node_features, np.float32)
    adj = np.ascontiguousarray(np.asarray(adjacency, np.float32))
    pos = np.asarray(node_positions, np.float32)
    vel = np.asarray(node_velocities, np.float32)
    types = np.asarray(node_types)
    P = int(num_ped_nodes)
    conf = np.asarray(conflict_softmax_group, np.float32)
    Wp = np.asarray(W_proj, np.float32)
    ws = np.asarray(w_score, np.float32)

    h = nf @ Wp                                     # [V, H] fp32
    li = h @ ws[:H]
    lj = h @ ws[H:2 * H]
    speed = np.sqrt((vel ** 2).sum(-1, keepdims=True))
    dirv = vel / (speed + EPS)
    ws4 = ws[2 * H:2 * H + 4]

    p2 = (pos.astype(np.float64) ** 2).sum(-1).astype(np.float32)
    v2 = (vel.astype(np.float64) ** 2).sum(-1).astype(np.float32)
    p2h, p2l = _split_hi_lo(p2)
    v2h, v2l = _split_hi_lo(v2)
    xh, xl = _split_hi_lo(pos[:, 0])
    yh, yl = _split_hi_lo(pos[:, 1])
    vxh, vxl = _split_hi_lo(vel[:, 0])
    vyh, vyl = _split_hi_lo(vel[:, 1])

    # K=10 hi/lo layout (feats side / lhsT side):
    #  [q2_hi, q2_lo, xh, xl, xh, yh, yl, yh, 1, 1]
    #  [1, 1, -2xh_i, -2xh_i, -2xl_i, -2yh_i, -2yh_i, -2yl_i, q2_hi, q2_lo]
    fdva = np.zeros((96, V), np.float32)
    fdva[0] = p2h; fdva[1] = p2l
    fdva[2] = xh;  fdva[3] = xl;  fdva[4] = xh
    fdva[5] = yh;  fdva[6] = yl;  fdva[7] = yh
    fdva[8] = 1.0; fdva[9] = 1.0
    fdva[10] = 1.0
    fdva[32] = v2h; fdva[33] = v2l
    fdva[34] = vxh; fdva[35] = vxl; fdva[36] = vxh
    fdva[37] = vyh; fdva[38] = vyl; fdva[39] = vyh
    fdva[40] = 1.0; fdva[41] = 1.0
    fdva[42] = 1.0
    fdva[64] = dirv[:, 0]; fdva[65] = dirv[:, 1]
    fb = np.zeros((4, V), np.float32)
    fb[0] = 1.0
    fb[1] = lj
    fb[2] = dirv[:, 0]
    fb[3] = dirv[:, 1]

    g = np.arange(V) - P
    valid = (types == 1) & (g >= 0) & (g < G)
    gc = np.clip(g, 0, G - 1)
    jcols = np.arange(V - G, V)
    gc_j = np.clip(jcols - P, 0, G - 1)
    valid_j = valid[jcols]

    # per-row upper bound on post-leaky logits (fp64, provably >= rowmax)
    p64 = pos.astype(np.float64)
    v64 = vel.astype(np.float64)
    pn = np.sqrt((p64 ** 2).sum(-1))
    vn = np.sqrt((v64 ** 2).sum(-1))
    t_dist = np.maximum(0.0, float(ws4[0]) * (pn + pn.max()))
    t_vel = np.maximum(0.0, float(ws4[1]) * (vn + vn.max()))
    m_pre = (li.astype(np.float64) + lj.astype(np.float64).max()
             + t_dist + t_vel + abs(float(ws4[2]))
             + max(float(ws4[3]), 0.0))
    m_bound = np.where(m_pre >= 0, m_pre, SLOPE * m_pre) + 0.5

    ws4_tile = np.ascontiguousarray(
        np.broadcast_to(ws4[None, :], (128, 4)).astype(np.float32))

    in_maps = []
    for c in range(N_CORES):
        rows = np.arange(c * ROWS, (c + 1) * ROWS)
        ldva = np.zeros((96, ROWS), np.float32)
        ldva[0] = 1.0; ldva[1] = 1.0
        ldva[2] = -2.0 * xh[rows]; ldva[3] = -2.0 * xh[rows]
        ldva[4] = -2.0 * xl[rows]
        ldva[5] = -2.0 * yh[rows]; ldva[6] = -2.0 * yh[rows]
        ldva[7] = -2.0 * yl[rows]
        ldva[8] = p2h[rows]; ldva[9] = p2l[rows]
        ldva[10] = 2e-3
        ldva[32] = 1.0; ldva[33] = 1.0
        ldva[34] = -2.0 * vxh[rows]; ldva[35] = -2.0 * vxh[rows]
        ldva[36] = -2.0 * vxl[rows]
        ldva[37] = -2.0 * vyh[rows]; ldva[38] = -2.0 * vyh[rows]
        ldva[39] = -2.0 * vyl[rows]
        ldva[40] = v2h[rows]; ldva[41] = v2l[rows]
        ldva[42] = 1e-4
        ldva[64] = dirv[rows, 0]; ldva[65] = dirv[rows, 1]
        lb = np.zeros((4, ROWS), np.float32)
        lb[0] = li[rows]
        lb[1] = 1.0
        lb[2] = float(ws4[2]) * dirv[rows, 0]
        lb[3] = float(ws4[2]) * dirv[rows, 1]

        conf_rows = (conf[gc[rows]][:, gc_j]
                     * valid[rows][:, None] * valid_j[None, :]).astype(np.float32)
        negM = np.ascontiguousarray(
            (-m_bound[rows]).astype(np.float32).reshape(RT, 128).T)
        in_maps.append({
            "feats_dva": fdva,
            "featsb": fb,
            "lhsT_dva": ldva,
            "lhsTb": lb,
            "conf_rows": np.ascontiguousarray(conf_rows),
            "adj_rows": np.ascontiguousarray(adj[rows]),
            "negM": negM,
            "ws4": ws4_tile,
        })
    return in_maps, h, valid, gc, P


def kernel(**inputs):
    from concourse.bass_utils import run_bass_kernel_spmd

    if "nc" not in _cache:
        _cache["nc"] = _build_program()
    nc = _cache["nc"]

    in_maps, h, valid, gc, P = _host_prep(**inputs)

    trace = os.environ.get("KERNEL_TRACE", "0") == "1"
    res = run_bass_kernel_spmd(
        nc, in_maps, core_ids=list(range(N_CORES)), trace=trace)
    _cache["last_results"] = res

    attn = np.concatenate([r["attn_out"] for r in res.results], axis=0)
    phi_pl = np.concatenate([r["phi_out"] for r in res.results], axis=0)
    phi = np.ascontiguousarray(phi_pl.transpose(0, 2, 1))

    # exact-zero diagonal (reference computes p_i - p_i = 0 exactly; the
    # bilinear-form route leaves ~sqrt(cancellation) residue there)
    idx = np.arange(V)
    phi[idx, idx, 0] = 0.0
    phi[idx, idx, 1] = 0.0

    if P != V - G:  # defensive: conflict block position hardcoded for P=3072
        conf = np.asarray(inputs["conflict_softmax_group"], np.float32)
        nconf = np.where(valid[:, None] & valid[None, :],
                         conf[gc[:, None], gc[None, :]], 0.0)
        phi[:, :, 3] = nconf

    attended = (attn @ h).astype(np.float32)
    return attended, attn, phi


# revision 17
# speedup vs baseline: 1.0705x; 1.0360x over previous
"""Trainium2 Bass kernel for nn_EnhancedSpatialAttention.

Row-sharded across 8 NeuronCores: core c owns query rows [c*512, (c+1)*512).
Each core computes its rows of phi [512,4096,4] and attn [512,4096] on
device; attended = attn @ h (tiny [4096,16]) is a host epilogue on the
gathered attn. Total device I/O is ~52.4 MB/core, so the kernel targets the
~358 GB/s-per-core HBM roofline (~147 us).

All pairwise quantities are bilinear forms computed on the PE in float32r
(1 cycle/row). float32r rounds operands to 13 mantissa bits, so the
position/velocity operands are split hi/lo (hi keeps 11 bits => exactly
representable; lo rounds with ~2^-24 relative error) and the three O(x*y)
cross terms are packed into a single K=10 contraction:
  dist^2 = 1*|pj|^2(hi+lo) - 2x_i*x_j - 2y_i*y_j + |pi|^2(hi+lo)*1
Measured accuracy: max_abs ~5e-4 over values up to ~5000 (q99 rel 2e-6),
same as a full fp32 matmul, 4x faster.

Phase A per row-tile rt (128 rows) x 1024-column chunk:
  PE (float32r): psum_d (K=10), psum_v (K=10), psum_a = dir_i.dir_j (K=2),
      psum_b = li + lj + ws2*(dir_i.dir_j) (K=4, ws2 folded into lhsT)
  ACT: phi plane 0 = sqrt(psum_d), plane 1 = sqrt(psum_v). A tiny +delta
       K-row (2e-3 / 1e-4) keeps the psums strictly positive against the
       ~1e-3 fp32 cancellation residue near zero distances - ACT sqrt of
       any negative is NaN (probed). The diagonal is re-zeroed on the host.
  DVE: phi plane 2 = clip(psum_a, -1, 1)
       logits = (phi0*ws0) + psum_b ; logits += phi1*ws1
       conflict cols [3072:4096]: logits += ws3*conf_rows
  ACT: conf_rows -> phi plane 3 (cols 3072:4096; rest stays memset 0)
       logits = prelu(logits, 0.2)  (probed: Prelu honors alpha, Lrelu not)
  DMA: phi out planar [128, 4, 1024] chunks; host transposes [V,4,V] ->
       [V,V,4] (0.2 s) so every engine op reads/writes dense unit-stride

Phase B per row-tile:
  ACT: e = exp(logits + (-M)); M = host per-row upper bound (exact softmax:
       the exp scale cancels in w/sum(w))
  DVE: w = e*adj with fused row-sum (scalar_tensor_tensor accum_out)
       attn = w * reciprocal(wsum) -> DMA out

Softmax equivalence with the reference (softmax(logits+mask)*adj then
renormalized with +1e-8): masked entries are exactly 0 both ways, and the
+1e-8 terms are below fp32 resolution of the O(1) row sums, so both reduce
to e*adj / sum(e*adj).
"""
import sys, os

if "/opt/trn_rl_repo" not in sys.path:
    sys.path.insert(0, "/opt/trn_rl_repo")

import numpy as np

V = 4096
H = 16
G = 1024
N_CORES = 8
ROWS = V // N_CORES          # 512 rows per core
RT = ROWS // 128             # 4 row-tiles per core
EPS = 1e-8
SLOPE = 0.2

_cache = {}


def _split_hi_lo(x):
    """hi keeps 11 explicit mantissa bits (exact in 13-bit float32r)."""
    x = np.asarray(x, np.float32)
    hi = (x.view(np.uint32) & np.uint32(0xFFFFF000)).view(np.float32)
    return hi, (x - hi).astype(np.float32)


def _build_program():
    import concourse.bass as bass
    import concourse.bacc as bacc
    import concourse.tile as tile
    from concourse import mybir
    from contextlib import ExitStack

    f32 = mybir.dt.float32
    f32r = mybir.dt.float32r
    AF = mybir.ActivationFunctionType
    OP = mybir.AluOpType

    nc = bacc.Bacc("TRN2", target_bir_lowering=False, debug=False)

    # d-quantity rows 0:10 | v rows 32:42 | a rows 64:66 (matmul operands
    # must start at base partition 0/32/64)
    fdva_dram = nc.dram_tensor("feats_dva", [96, V], f32, kind="ExternalInput")
    fb_dram = nc.dram_tensor("featsb", [4, V], f32, kind="ExternalInput")
    ldva_dram = nc.dram_tensor("lhsT_dva", [96, ROWS], f32, kind="ExternalInput")
    lb_dram = nc.dram_tensor("lhsTb", [4, ROWS], f32, kind="ExternalInput")
    conf_dram = nc.dram_tensor("conf_rows", [ROWS, G], f32, kind="ExternalInput")
    adj_dram = nc.dram_tensor("adj_rows", [ROWS, V], f32, kind="ExternalInput")
    negM_dram = nc.dram_tensor("negM", [128, RT], f32, kind="ExternalInput")
    ws_dram = nc.dram_tensor("ws4", [128, 4], f32, kind="ExternalInput")

    phi_dram = nc.dram_tensor("phi_out", [ROWS, 4, V], f32, kind="ExternalOutput")
    attn_dram = nc.dram_tensor("attn_out", [ROWS, V], f32, kind="ExternalOutput")

    CH = 1024                    # column chunk == phi quarter width
    NCH = V // CH

    with tile.TileContext(nc) as tc:
        with ExitStack() as ctx:
            persist = ctx.enter_context(tc.tile_pool(name="persist", bufs=1))
            psum_pool = ctx.enter_context(
                tc.tile_pool(name="psum", bufs=1, space="PSUM"))

            logits = [persist.tile([128, V], f32, tag=f"logits{rt}",
                                   name=f"logits{rt}") for rt in range(RT)]
            negM = persist.tile([128, RT], f32, tag="negM", name="negM")
            ws4 = persist.tile([128, 4], f32, tag="ws4", name="ws4")
            nc.sync.dma_start(negM[:], negM_dram[:])
            nc.sync.dma_start(ws4[:], ws_dram[:])

            # ---------------- Phase A ----------------
            with ExitStack() as ctxA:
                pa = ctxA.enter_context(tc.tile_pool(name="pa", bufs=1))
                confp = ctxA.enter_context(tc.tile_pool(name="confp", bufs=2))

                fdva = pa.tile([96, V], f32r, tag="fdva", name="fdva")
                fb = pa.tile([4, V], f32r, tag="fb", name="fb")
                ldva = pa.tile([96, ROWS], f32r, tag="ldva", name="ldva")
                lb = pa.tile([4, ROWS], f32r, tag="lb", name="lb")
                nc.gpsimd.dma_start(fdva[:], fdva_dram[:])
                nc.gpsimd.dma_start(ldva[:], ldva_dram[:])
                nc.gpsimd.dma_start(fb[:], fb_dram[:])
                nc.gpsimd.dma_start(lb[:], lb_dram[:])

                # phi quarter buffers: even chunks -> phiA, odd -> phiB.
                # lane 3 of phiA stays all-zero; phiB's lane 3 is rewritten
                # with conf on chunk 3 and re-zeroed before chunk 1 reuses it.
                phiA = pa.tile([128, 4, CH], f32, tag="phiA", name="phiA")
                phiB = pa.tile([128, 4, CH], f32, tag="phiB", name="phiB")
                phiC = pa.tile([128, 4, CH], f32, tag="phiC", name="phiC")
                nc.vector.memset(phiA[:, 3, :], 0.0)
                nc.vector.memset(phiC[:, 3, :], 0.0)

                for rt in range(RT):
                    sA = logits[rt]
                    rsl = slice(rt * 128, (rt + 1) * 128)
                    conf_t = confp.tile([128, G], f32, tag="conf", name="conf")
                    nc.scalar.dma_start(conf_t[:], conf_dram[rsl, :])
                    qmap = [phiA, phiB, phiC, phiB]
                    for c in range(NCH):
                        quarter = qmap[c]
                        if c == 1:
                            # clear stale conflict values from last rt's c==3
                            nc.gpsimd.memset(phiB[:, 3, :], 0.0)
                        ps_d = psum_pool.tile([128, CH], f32, tag="psd", name="psd")
                        ps_v = psum_pool.tile([128, CH], f32, tag="psv", name="psv")
                        ps_a = psum_pool.tile([128, CH], f32, tag="psa", name="psa")
                        ps_b = psum_pool.tile([128, CH], f32, tag="psb", name="psb")
                        for s in range(2):
                            cs = slice(c * CH + s * 512, c * CH + s * 512 + 512)
                            half = slice(s * 512, s * 512 + 512)
                            nc.tensor.matmul(ps_d[:, half], ldva[0:11, rsl],
                                             fdva[0:11, cs], start=True, stop=True)
                            nc.tensor.matmul(ps_v[:, half], ldva[32:43, rsl],
                                             fdva[32:43, cs], start=True, stop=True)
                            nc.tensor.matmul(ps_a[:, half], ldva[64:66, rsl],
                                             fdva[64:66, cs], start=True, stop=True)
                            nc.tensor.matmul(ps_b[:, half], lb[:, rsl],
                                             fb[:, cs], start=True, stop=True)
                        lane = lambda k: quarter[:, k, :]
                        nc.scalar.activation(lane(0), ps_d[:], AF.Sqrt)
                        nc.scalar.activation(lane(1), ps_v[:], AF.Sqrt)
                        nc.vector.tensor_scalar(
                            out=lane(2), in0=ps_a[:], scalar1=1.0, scalar2=-1.0,
                            op0=OP.min, op1=OP.max)
                        cols = slice(c * CH, (c + 1) * CH)
                        nc.vector.scalar_tensor_tensor(
                            out=sA[:, cols], in0=lane(0),
                            scalar=ws4[:, 0:1], in1=ps_b[:],
                            op0=OP.mult, op1=OP.add)
                        nc.vector.scalar_tensor_tensor(
                            out=sA[:, cols], in0=lane(1),
                            scalar=ws4[:, 1:2], in1=sA[:, cols],
                            op0=OP.mult, op1=OP.add)
                        if c == 3:
                            # conflict block lives in cols [V-G, V) == chunk 3
                            nc.scalar.activation(lane(3), conf_t[:], AF.Copy)
                            nc.vector.scalar_tensor_tensor(
                                out=sA[:, cols], in0=conf_t[:],
                                scalar=ws4[:, 3:4], in1=sA[:, cols],
                                op0=OP.mult, op1=OP.add)
                        r0 = rt * 128
                        nc.sync.dma_start(
                            phi_dram[r0:r0 + 128, :, c * CH:(c + 1) * CH],
                            quarter[:])
                    # leaky relu in place (Prelu honors alpha; Lrelu's is baked)
                    nc.scalar.activation(
                        logits[rt][:], logits[rt][:], AF.Prelu, alpha=SLOPE)

            # ---------------- Phase B ----------------
            with ExitStack() as ctxB:
                pb = ctxB.enter_context(tc.tile_pool(name="pb", bufs=2))
                HV = V // 2
                for rt in range(RT):
                    r0 = rt * 128
                    adj_t = pb.tile([128, V], f32, tag="adj", name="adj_t")
                    e_t = pb.tile([128, V], f32, tag="e", name="e_t")
                    w_t = pb.tile([128, V], f32, tag="w", name="w_t", bufs=3)
                    wsum_h = pb.tile([128, 2], f32, tag="wsumh", name="wsum_h")
                    for hh in range(2):
                        cl = slice(hh * HV, (hh + 1) * HV)
                        nc.scalar.dma_start(
                            adj_t[:, cl], adj_dram[r0:r0 + 128, cl])
                        nc.scalar.activation(
                            e_t[:, cl], logits[rt][:, cl], AF.Exp,
                            bias=negM[:, rt:rt + 1], scale=1.0)
                        nc.vector.scalar_tensor_tensor(
                            out=w_t[:, cl], in0=e_t[:, cl], scalar=1.0,
                            in1=adj_t[:, cl], op0=OP.mult, op1=OP.mult,
                            accum_out=wsum_h[:, hh:hh + 1])
                    wsum = pb.tile([128, 1], f32, tag="wsum", name="wsum")
                    nc.vector.tensor_tensor(
                        out=wsum[:], in0=wsum_h[:, 0:1], in1=wsum_h[:, 1:2],
                        op=OP.add)
                    rcp = pb.tile([128, 1], f32, tag="rcp", name="rcp")
                    nc.vector.reciprocal(rcp[:], wsum[:])
                    for hh in range(2):
                        cl = slice(hh * HV, (hh + 1) * HV)
                        nc.vector.tensor_scalar_mul(
                            out=w_t[:, cl], in0=w_t[:, cl], scalar1=rcp[:])
                        nc.sync.dma_start(
                            attn_dram[r0:r0 + 128, cl], w_t[:, cl])

    nc.compile()
    return nc


def _host_prep(node_features, adjacency, node_positions, node_velocities,
               node_types, num_ped_nodes, conflict_softmax_group,
               W_proj, w_score):
    nf = np.asarray(node_features, np.float32)
    adj = np.ascontiguousarray(np.asarray(adjacency, np.float32))
    pos = np.asarray(node_positions, np.float32)
    vel = np.asarray(node_velocities, np.float32)
    types = np.asarray(node_types)
    P = int(num_ped_nodes)
    conf = np.asarray(conflict_softmax_group, np.float32)
    Wp = np.asarray(W_proj, np.float32)
    ws = np.asarray(w_score, np.float32)

    h = nf @ Wp                                     # [V, H] fp32
    li = h @ ws[:H]
    lj = h @ ws[H:2 * H]
    speed = np.sqrt((vel ** 2).sum(-1, keepdims=True))
    dirv = vel / (speed + EPS)
    ws4 = ws[2 * H:2 * H + 4]

    p2 = (pos.astype(np.float64) ** 2).sum(-1).astype(np.float32)
    v2 = (vel.astype(np.float64) ** 2).sum(-1).astype(np.float32)
    p2h, p2l = _split_hi_lo(p2)
    v2h, v2l = _split_hi_lo(v2)
    xh, xl = _split_hi_lo(pos[:, 0])
    yh, yl = _split_hi_lo(pos[:, 1])
    vxh, vxl = _split_hi_lo(vel[:, 0])
    vyh, vyl = _split_hi_lo(vel[:, 1])

    # K=10 hi/lo layout (feats side / lhsT side):
    #  [q2_hi, q2_lo, xh, xl, xh, yh, yl, yh, 1, 1]
    #  [1, 1, -2xh_i, -2xh_i, -2xl_i, -2yh_i, -2yh_i, -2yl_i, q2_hi, q2_lo]
    fdva = np.zeros((96, V), np.float32)
    fdva[0] = p2h; fdva[1] = p2l
    fdva[2] = xh;  fdva[3] = xl;  fdva[4] = xh
    fdva[5] = yh;  fdva[6] = yl;  fdva[7] = yh
    fdva[8] = 1.0; fdva[9] = 1.0
    fdva[10] = 1.0
    fdva[32] = v2h; fdva[33] = v2l
    fdva[34] = vxh; fdva[35] = vxl; fdva[36] = vxh
    fdva[37] = vyh; fdva[38] = vyl; fdva[39] = vyh
    fdva[40] = 1.0; fdva[41] = 1.0
    fdva[42] = 1.0
    fdva[64] = dirv[:, 0]; fdva[65] = dirv[:, 1]
    fb = np.zeros((4, V), np.float32)
    fb[0] = 1.0
    fb[1] = lj
    fb[2] = dirv[:, 0]
    fb[3] = dirv[:, 1]

    g = np.arange(V) - P
    valid = (types == 1) & (g >= 0) & (g < G)
    gc = np.clip(g, 0, G - 1)
    jcols = np.arange(V - G, V)
    gc_j = np.clip(jcols - P, 0, G - 1)
    valid_j = valid[jcols]

    # per-row upper bound on post-leaky logits (fp64, provably >= rowmax)
    p64 = pos.astype(np.float64)
    v64 = vel.astype(np.float64)
    pn = np.sqrt((p64 ** 2).sum(-1))
    vn = np.sqrt((v64 ** 2).sum(-1))
    t_dist = np.maximum(0.0, float(ws4[0]) * (pn + pn.max()))
    t_vel = np.maximum(0.0, float(ws4[1]) * (vn + vn.max()))
    m_pre = (li.astype(np.float64) + lj.astype(np.float64).max()
             + t_dist + t_vel + abs(float(ws4[2]))
             + max(float(ws4[3]), 0.0))
    m_bound = np.where(m_pre >= 0, m_pre, SLOPE * m_pre) + 0.5

    ws4_tile = np.ascontiguousarray(
        np.broadcast_to(ws4[None, :], (128, 4)).astype(np.float32))

    in_maps = []
    for c in range(N_CORES):
        rows = np.arange(c * ROWS, (c + 1) * ROWS)
        ldva = np.zeros((96, ROWS), np.float32)
        ldva[0] = 1.0; ldva[1] = 1.0
        ldva[2] = -2.0 * xh[rows]; ldva[3] = -2.0 * xh[rows]
        ldva[4] = -2.0 * xl[rows]
        ldva[5] = -2.0 * yh[rows]; ldva[6] = -2.0 * yh[rows]
        ldva[7] = -2.0 * yl[rows]
        ldva[8] = p2h[rows]; ldva[9] = p2l[rows]
        ldva[10] = 2e-3
        ldva[32] = 1.0; ldva[33] = 1.0
        ldva[34] = -2.0 * vxh[rows]; ldva[35] = -2.0 * vxh[rows]
        ldva[36] = -2.0 * vxl[rows]
        ldva[37] = -2.0 * vyh[rows]; ldva[38] = -2.0 * vyh[rows]
        ldva[39] = -2.0 * vyl[rows]
        ldva[40] = v2h[rows]; ldva[41] = v2l[rows]
        ldva[42] = 1e-4
        ldva[64] = dirv[rows, 0]; ldva[65] = dirv[rows, 1]
        lb = np.zeros((4, ROWS), np.float32)
        lb[0] = li[rows]
        lb[1] = 1.0
        lb[2] = float(ws4[2]) * dirv[rows, 0]
        lb[3] = float(ws4[2]) * dirv[rows, 1]

        conf_rows = (conf[gc[rows]][:, gc_j]
                     * valid[rows][:, None] * valid_j[None, :]).astype(np.float32)
        negM = np.ascontiguousarray(
            (-m_bound[rows]).astype(np.float32).reshape(RT, 128).T)
        in_maps.append({
            "feats_dva": fdva,
            "featsb": fb,
            "lhsT_dva": ldva,
            "lhsTb": lb,
            "conf_rows": np.ascontiguousarray(conf_rows),
            "adj_rows": np.ascontiguousarray(adj[rows]),
            "negM": negM,
            "ws4": ws4_tile,
        })
    return in_maps, h, valid, gc, P


def kernel(**inputs):
    from concourse.bass_utils import run_bass_kernel_spmd

    if "nc" not in _cache:
        _cache["nc"] = _build_program()
    nc = _cache["nc"]

    in_maps, h, valid, gc, P = _host_prep(**inputs)

    trace = os.environ.get("KERNEL_TRACE", "0") == "1"
    res = run_bass_kernel_spmd(
        nc, in_maps, core_ids=list(range(N_CORES)), trace=trace)
    _cache["last_results"] = res

    attn = np.concatenate([r["attn_out"] for r in res.results], axis=0)
    phi_pl = np.concatenate([r["phi_out"] for r in res.results], axis=0)
    phi = np.ascontiguousarray(phi_pl.transpose(0, 2, 1))

    # exact-zero diagonal (reference computes p_i - p_i = 0 exactly; the
    # bilinear-form route leaves ~sqrt(cancellation) residue there)
    idx = np.arange(V)
    phi[idx, idx, 0] = 0.0
    phi[idx, idx, 1] = 0.0

    if P != V - G:  # defensive: conflict block position hardcoded for P=3072
        conf = np.asarray(inputs["conflict_softmax_group"], np.float32)
        nconf = np.where(valid[:, None] & valid[None, :],
                         conf[gc[:, None], gc[None, :]], 0.0)
        phi[:, :, 3] = nconf

    attended = (attn @ h).astype(np.float32)
    return attended, attn, phi
